# revision 3
# baseline (speedup 1.0000x reference)
"""Trainium2 Bass kernel for: 3x3 conv (reflect pad) + BatchNorm + LeakyReLU + mask.

Input  x:    (1, 64, 512, 512) f32
       W:    (128, 64, 3, 3)   f32
       gamma/beta/mean/var: (128,) f32
       mask: (1, 128, 512, 512) int32 (0/1)
Output (1, 128, 512, 512) f32

Strategy (8 cores, SPMD):
  - Shard H spatially: core c computes output rows [64c, 64c+64).
  - Each core works on TWO bf16 copies of its 66-row input slab stacked into a
    [128, 66*514] SBUF image (copy1 = copy0 shifted down one row).  A K=128
    matmul against stacked weights computes two conv taps at once:
      partitions   0..63 : channel ci at row y+dy      (copy0)
      partitions 64..127 : channel ci at row y+dy+1    (copy1)
  - 9 taps -> 6 matmuls per output row: 3 "pair" matmuls (dy=0&1, dx=0..2)
    and, per row pair, 6 K=64 dy=2 matmuls on disjoint partition halves that
    execute concurrently -> 9 N=512 matmul slots per 2 rows (PE-optimal).
  - HBM traffic minimized: copy0 is shipped once from HBM (4.3 MB); copy1 is
    rebuilt on-chip with SBUF->SBUF shift DMAs (only its first HEAD rows ship
    from HBM to cut the startup dependency chain).  Output is stored as bf16
    (8 MB instead of 16) and widened to f32 on the host.  Weights go in one
    packed DMA.  Total HBM/core ~17 MB (~47 us) < PE time (~62 us).
  - PE pre-warm: 16 throwaway matmuls on a zeroed tile while inputs stream in,
    so the HAM clock gate (1.2 -> 2.4 GHz after ~3.4 us busy) is already
    released when the real matmul stream starts.
  - Epilogue = ACT Lrelu(psum*scale+shift) -> bf16, DVE multiply by uint8 mask.
"""

import numpy as np
import ml_dtypes

import concourse.bacc as bacc
import concourse.bass as bass
import concourse.mybir as mybir
import concourse.tile as tile
from concourse.bass_utils import run_bass_kernel_spmd

bf16 = ml_dtypes.bfloat16

N_CORES = 8
C_IN = 64
C_OUT = 128
H = 512
W_IMG = 512
HS = H // N_CORES            # 64 output rows per core
WP = W_IMG + 2               # 514 padded columns
NROW = HS + 2                # 66 input rows per core (64 + 1-row halo each side)
FREE = NROW * WP             # per-partition free elems of the x image
HEAD = 10                    # copy1 rows shipped from HBM (rest shift-copied)
G = 8                        # output rows per mask load chunk
SG = 4                       # output rows per store tile
LEAK = 0.01
EPS = 1e-5

# copy0 HBM load chunks (input row ranges); sized so each on-chip copy chunk
# depends on exactly one load chunk
XCH = [(0, 5), (5, 11), (11, 21), (21, 31), (31, 41), (41, 51), (51, 61), (61, 66)]
# on-chip shift-copy chunks: copy1 rows [r0,r1) built from copy0 rows [r0+1,r1+1)
CCH = [(10, 20), (20, 30), (30, 40), (40, 50), (50, 60), (60, 65)]

_CACHE = {}
LAST_RESULTS = None          # BassKernelResults of the last run (for test.py)


def _build_program(hw_lrelu: bool = True) -> bass.Bass:
    """hw_lrelu=True uses the ACT engine's native Lrelu (not implemented in
    CoreSim); False uses an Identity + DVE max(z*a, z) fallback."""
    nc = bacc.Bacc("TRN2", target_bir_lowering=False, debug=False,
                   num_devices=N_CORES)
    f32 = mybir.dt.float32
    bf = mybir.dt.bfloat16
    u8 = mybir.dt.uint8

    xs_d = nc.dram_tensor("xs", [128, FREE], bf, kind="ExternalInput")
    wp_d = nc.dram_tensor("wp", [128, 6 * C_OUT], bf, kind="ExternalInput")
    bn_d = nc.dram_tensor("bn", [C_OUT, 2], f32, kind="ExternalInput")
    mk_d = nc.dram_tensor("msk", [C_OUT, HS * W_IMG], u8, kind="ExternalInput")
    out_d = nc.dram_tensor("out", [C_OUT, HS * W_IMG], bf, kind="ExternalOutput")

    with tile.TileContext(nc) as tc:
        with tc.tile_pool(name="const", bufs=1) as cpool, \
             tc.tile_pool(name="xp", bufs=1) as xpool, \
             tc.tile_pool(name="zp", bufs=4) as zpool, \
             tc.tile_pool(name="op", bufs=3) as opool, \
             tc.tile_pool(name="ps", bufs=8, space="PSUM") as ppool:

            wts = cpool.tile([128, 6 * C_OUT], bf, name="wts", tag="wts")
            bn = cpool.tile([C_OUT, 2], f32, name="bn_t", tag="bn_t")
            mt = cpool.tile([C_OUT, HS * W_IMG], u8, name="mt", tag="mt")
            warm = cpool.tile([128, 128 + W_IMG], bf, name="warm", tag="warm")
            xs = xpool.tile([128, FREE], bf, name="xs_t", tag="xs_t")

            # PE pre-warm on a zeroed tile (see module docstring); the warm
            # psum is just the first rotation of the shared "pst" buffers
            nc.gpsimd.memset(warm[:], 0.0)
            psw = ppool.tile([C_OUT, W_IMG], f32, name="psw", tag="pst")
            for _ in range(8):
                nc.tensor.matmul(psw[:], warm[:, 0:128], warm[:, 128:128 + W_IMG],
                                 start=True, stop=True)

            # scalar(qAct) HWDGE ring: small latency-critical tiles up front
            nc.scalar.dma_start(out=wts[:], in_=wp_d[:])
            nc.scalar.dma_start(out=bn[:], in_=bn_d[:])
            nc.scalar.dma_start(out=xs[64:128, 0:HEAD * WP],
                                in_=xs_d[64:128, 0:HEAD * WP])
            for g in (0, 1):
                nc.scalar.dma_start(
                    out=mt[:, g * G * W_IMG:(g + 1) * G * W_IMG],
                    in_=mk_d[:, g * G * W_IMG:(g + 1) * G * W_IMG])

            # sync(qSP) HWDGE ring: bulk copy0 stream, then remaining masks,
            # then (inside the loop) the output stores
            for r0, r1 in XCH:
                nc.sync.dma_start(out=xs[0:64, r0 * WP:r1 * WP],
                                  in_=xs_d[0:64, r0 * WP:r1 * WP])
            for g in range(2, HS // G):
                nc.sync.dma_start(
                    out=mt[:, g * G * W_IMG:(g + 1) * G * W_IMG],
                    in_=mk_d[:, g * G * W_IMG:(g + 1) * G * W_IMG])

            # gpsimd SWDGE: on-chip one-row-down shift copies building copy1
            for r0, r1 in CCH:
                nc.gpsimd.dma_start(out=xs[64:128, r0 * WP:r1 * WP],
                                    in_=xs[0:64, (r0 + 1) * WP:(r1 + 1) * WP])

            ot = None

            def epilogue(y, pst):
                seg = slice((y % SG) * W_IMG, (y % SG + 1) * W_IMG)
                mseg = slice(y * W_IMG, (y + 1) * W_IMG)
                if hw_lrelu:
                    nc.scalar.activation(
                        ot[:, seg], pst[:],
                        mybir.ActivationFunctionType.Lrelu,
                        bias=bn[:, 1:2], scale=bn[:, 0:1], alpha=LEAK)
                else:
                    zt = zpool.tile([C_OUT, W_IMG], f32, name="zt", tag="zt")
                    nc.scalar.activation(
                        zt[:], pst[:],
                        mybir.ActivationFunctionType.Identity,
                        bias=bn[:, 1:2], scale=bn[:, 0:1])
                    nc.vector.scalar_tensor_tensor(
                        ot[:, seg], zt[:], LEAK, zt[:],
                        op0=mybir.AluOpType.mult, op1=mybir.AluOpType.max)
                nc.vector.tensor_tensor(ot[:, seg], ot[:, seg], mt[:, mseg],
                                        op=mybir.AluOpType.mult)
                if y % SG == SG - 1:
                    d0 = (y - SG + 1) * W_IMG
                    nc.sync.dma_start(out=out_d[:, d0:d0 + SG * W_IMG],
                                      in_=ot[:])

            # rows processed in pairs: the three K=64 dy=2 taps of row y run
            # on PE rows 0-63 (copy0) while row y+1's run on rows 64-127
            # (copy1, same flat offset) — disjoint row-groups + separate PSUM
            # banks execute concurrently, so 2 rows cost 9 MM slots, not 12
            for y in range(0, HS, 2):
                if y % SG == 0:
                    ot = opool.tile([C_OUT, SG * W_IMG], bf, name="ot", tag="ot")
                ps_a = ppool.tile([C_OUT, W_IMG], f32, name="ps_a", tag="pst")
                ps_b = ppool.tile([C_OUT, W_IMG], f32, name="ps_b", tag="pst")
                for yy, ps in ((y, ps_a), (y + 1, ps_b)):
                    for dx in range(3):
                        off = yy * WP + dx
                        nc.tensor.matmul(ps[:], wts[:, dx * 128:dx * 128 + 128],
                                         xs[:, off:off + W_IMG],
                                         start=(dx == 0), stop=False)
                for dx in range(3):
                    off = (y + 2) * WP + dx
                    j = 3 + dx
                    nc.tensor.matmul(ps_a[:], wts[0:64, j * 128:j * 128 + 128],
                                     xs[0:64, off:off + W_IMG],
                                     start=False, stop=(dx == 2))
                    nc.tensor.matmul(ps_b[:], wts[64:128, j * 128:j * 128 + 128],
                                     xs[64:128, off:off + W_IMG],
                                     start=False, stop=(dx == 2))
                epilogue(y, ps_a)
                epilogue(y + 1, ps_b)
    nc.compile()
    return nc


def _get_program(hw_lrelu: bool = True) -> bass.Bass:
    key = ("nc", hw_lrelu)
    if key not in _CACHE:
        _CACHE[key] = _build_program(hw_lrelu)
    return _CACHE[key]


def make_in_maps(x, W, gamma, beta, mean, var, mask):
    """Host-side shard/pack of full inputs into per-core in_maps."""
    x = np.asarray(x, np.float32)
    W = np.asarray(W, np.float32)
    gamma = np.asarray(gamma, np.float32)
    beta = np.asarray(beta, np.float32)
    mean = np.asarray(mean, np.float32)
    var = np.asarray(var, np.float32)
    mask = np.asarray(mask)

    xp = np.pad(x[0], ((0, 0), (1, 1), (1, 1)), mode="reflect")   # [64,514,514]
    xpb = xp.astype(bf16)

    wp = np.zeros((6, 128, C_OUT), np.float32)
    for dx in range(3):
        wp[dx, 0:64] = W[:, :, 0, dx].reshape(C_OUT, C_IN).T
        wp[dx, 64:128] = W[:, :, 1, dx].reshape(C_OUT, C_IN).T
        # dy=2 taps duplicated: rows 0-63 serve even rows via copy0,
        # rows 64-127 serve odd rows via copy1 (concurrent row-tiled MMs)
        wp[3 + dx, 0:64] = W[:, :, 2, dx].reshape(C_OUT, C_IN).T
        wp[3 + dx, 64:128] = W[:, :, 2, dx].reshape(C_OUT, C_IN).T
    wpk = np.ascontiguousarray(
        wp.transpose(1, 0, 2)).reshape(128, 6 * C_OUT).astype(bf16)

    inv = 1.0 / np.sqrt(var + EPS)
    bn = np.stack([gamma * inv, beta - mean * gamma * inv],
                  axis=1).astype(np.float32)                      # [128,2]

    m8 = mask[0].astype(np.uint8)                                 # [128,512,512]

    in_maps = []
    for c in range(N_CORES):
        S = np.ascontiguousarray(xpb[:, HS * c:HS * c + NROW, :])  # [64,66,514]
        xs_c = np.zeros((128, FREE), bf16)
        xs_c[0:64] = S.reshape(C_IN, FREE)
        # only the first HEAD rows of copy1 ship from HBM; the device never
        # reads the rest of this half (it is rebuilt on-chip)
        xs_c[64:128, 0:HEAD * WP] = np.ascontiguousarray(
            S[:, 1:1 + HEAD, :]).reshape(C_IN, HEAD * WP)
        mk_c = np.ascontiguousarray(
            m8[:, HS * c:HS * c + HS, :]).reshape(C_OUT, HS * W_IMG)
        in_maps.append(dict(xs=xs_c, wp=wpk, bn=bn, msk=mk_c))
    return in_maps


def kernel(x, W, gamma, beta, mean, var, mask, _trace=False):
    global LAST_RESULTS
    nc = _get_program()
    in_maps = make_in_maps(x, W, gamma, beta, mean, var, mask)
    res = run_bass_kernel_spmd(nc, in_maps, list(range(N_CORES)), trace=_trace)
    LAST_RESULTS = res
    out = np.empty((1, C_OUT, H, W_IMG), np.float32)
    for c in range(N_CORES):
        out[0, :, HS * c:HS * c + HS, :] = np.asarray(
            res.results[c]["out"]).astype(np.float32).reshape(C_OUT, HS, W_IMG)
    return out


# revision 6
# speedup vs baseline: 1.0919x; 1.0919x over previous
"""Trainium2 Bass kernel for: 3x3 conv (reflect pad) + BatchNorm + LeakyReLU + mask.

Input  x:    (1, 64, 512, 512) f32
       W:    (128, 64, 3, 3)   f32
       gamma/beta/mean/var: (128,) f32
       mask: (1, 128, 512, 512) int32 (0/1)
Output (1, 128, 512, 512) f32

Strategy (8 cores, SPMD):
  - Shard H spatially: core c computes output rows [64c, 64c+64).
  - Each core works on TWO bf16 copies of its 66-row input slab stacked into a
    [128, 66*514] SBUF image (copy1 = copy0 shifted down one row).  A K=128
    matmul against stacked weights computes two conv taps at once:
      partitions   0..63 : channel ci at row y+dy      (copy0)
      partitions 64..127 : channel ci at row y+dy+1    (copy1)
  - 9 taps -> 6 matmuls per output row: 3 "pair" matmuls (dy=0&1, dx=0..2)
    and, per row pair, 6 K=64 dy=2 matmuls on disjoint partition halves that
    execute concurrently -> 9 N=512 matmul slots per 2 rows (PE-optimal).
  - HBM traffic minimized: copy0 is shipped once from HBM (4.3 MB); copy1 is
    rebuilt on-chip with SBUF->SBUF shift DMAs (only its first HEAD rows ship
    from HBM to cut the startup dependency chain).  Output is stored as bf16
    (8 MB instead of 16) and widened to f32 on the host.  Weights go in one
    packed DMA.  Total HBM/core ~17 MB (~47 us) < PE time (~62 us).
  - PE pre-warm: 16 throwaway matmuls on a zeroed tile while inputs stream in,
    so the HAM clock gate (1.2 -> 2.4 GHz after ~3.4 us busy) is already
    released when the real matmul stream starts.
  - Epilogue = ACT Lrelu(psum*scale+shift) -> bf16, DVE multiply by uint8 mask.
"""

import numpy as np
import ml_dtypes

import concourse.bacc as bacc
import concourse.bass as bass
import concourse.mybir as mybir
import concourse.tile as tile
from concourse.bass_utils import run_bass_kernel_spmd

bf16 = ml_dtypes.bfloat16

N_CORES = 8
C_IN = 64
C_OUT = 128
H = 512
W_IMG = 512
HS = H // N_CORES            # 64 output rows per core
WP = W_IMG + 2               # 514 padded columns
NROW = HS + 2                # 66 input rows per core (64 + 1-row halo each side)
FREE = NROW * WP             # per-partition free elems of the x image
HEAD = 30                    # copy1 rows shipped from HBM (rest shift-copied)
G = 8                        # output rows per mask load chunk
SG = 4                       # output rows per store tile
LEAK = 0.01
EPS = 1e-5

# copy0 HBM load chunks (input row ranges); sized so each on-chip copy chunk
# depends on exactly one load chunk
XCH = [(0, 5), (5, 11), (11, 21), (21, 31), (31, 41), (41, 51), (51, 61), (61, 66)]
# copy1 HBM head chunks (first chunk tiny so the first matmul starts early)
HCH = [(0, 3), (3, 15), (15, 30)]
# on-chip shift-copy chunks: copy1 rows [r0,r1) built from copy0 rows [r0+1,r1+1);
# issued on the sync ring AFTER the x stream, when their sources have landed
CCH = [(30, 40), (40, 50), (50, 60), (60, 65)]

_CACHE = {}
LAST_RESULTS = None          # BassKernelResults of the last run (for test.py)


def _build_program(hw_lrelu: bool = True) -> bass.Bass:
    """hw_lrelu=True uses the ACT engine's native Lrelu (not implemented in
    CoreSim); False uses an Identity + DVE max(z*a, z) fallback."""
    nc = bacc.Bacc("TRN2", target_bir_lowering=False, debug=False,
                   num_devices=N_CORES)
    f32 = mybir.dt.float32
    bf = mybir.dt.bfloat16
    u8 = mybir.dt.uint8

    xs_d = nc.dram_tensor("xs", [128, FREE], bf, kind="ExternalInput")
    wp_d = nc.dram_tensor("wp", [128, 6 * C_OUT], bf, kind="ExternalInput")
    bn_d = nc.dram_tensor("bn", [C_OUT, 2], f32, kind="ExternalInput")
    mk_d = nc.dram_tensor("msk", [C_OUT, HS * W_IMG], u8, kind="ExternalInput")
    out_d = nc.dram_tensor("out", [C_OUT, HS * W_IMG], bf, kind="ExternalOutput")

    with tile.TileContext(nc) as tc:
        with tc.tile_pool(name="const", bufs=1) as cpool, \
             tc.tile_pool(name="xp", bufs=1) as xpool, \
             tc.tile_pool(name="zp", bufs=4) as zpool, \
             tc.tile_pool(name="op", bufs=3) as opool, \
             tc.tile_pool(name="ps", bufs=8, space="PSUM") as ppool:

            wts = cpool.tile([128, 6 * C_OUT], bf, name="wts", tag="wts")
            bn = cpool.tile([C_OUT, 2], f32, name="bn_t", tag="bn_t")
            mt = cpool.tile([C_OUT, HS * W_IMG], u8, name="mt", tag="mt")
            warm = cpool.tile([128, 128 + W_IMG], bf, name="warm", tag="warm")
            xs = xpool.tile([128, FREE], bf, name="xs_t", tag="xs_t")

            # PE pre-warm on a zeroed tile (see module docstring); the warm
            # psum is just the first rotation of the shared "pst" buffers
            nc.gpsimd.memset(warm[:], 0.0)
            psw = ppool.tile([C_OUT, W_IMG], f32, name="psw", tag="pst")
            for _ in range(8):
                nc.tensor.matmul(psw[:], warm[:, 0:128], warm[:, 128:128 + W_IMG],
                                 start=True, stop=True)

            # scalar(qAct) HWDGE ring: weights + copy1 head up front, then
            # (inside the loop) the output stores
            nc.scalar.dma_start(out=wts[:], in_=wp_d[:])
            nc.scalar.dma_start(out=bn[:], in_=bn_d[:])
            for r0, r1 in HCH:
                nc.scalar.dma_start(out=xs[64:128, r0 * WP:r1 * WP],
                                    in_=xs_d[64:128, r0 * WP:r1 * WP])

            # sync(qSP) HWDGE ring: bulk copy0 stream, then the on-chip
            # one-row-down shift copies building the tail of copy1 (their
            # x-chunk deps have landed by then, so they never block the ring)
            for r0, r1 in XCH:
                nc.sync.dma_start(out=xs[0:64, r0 * WP:r1 * WP],
                                  in_=xs_d[0:64, r0 * WP:r1 * WP])
            for r0, r1 in CCH:
                nc.sync.dma_start(out=xs[64:128, r0 * WP:r1 * WP],
                                  in_=xs[0:64, (r0 + 1) * WP:(r1 + 1) * WP])

            # gpsimd SWDGE: mask chunks (separate queue so they never sit
            # behind the x stream or ahead of the stores)
            for g in range(HS // G):
                nc.gpsimd.dma_start(
                    out=mt[:, g * G * W_IMG:(g + 1) * G * W_IMG],
                    in_=mk_d[:, g * G * W_IMG:(g + 1) * G * W_IMG])

            ot = None

            def epilogue(y, pst):
                seg = slice((y % SG) * W_IMG, (y % SG + 1) * W_IMG)
                mseg = slice(y * W_IMG, (y + 1) * W_IMG)
                if hw_lrelu:
                    nc.scalar.activation(
                        ot[:, seg], pst[:],
                        mybir.ActivationFunctionType.Lrelu,
                        bias=bn[:, 1:2], scale=bn[:, 0:1], alpha=LEAK)
                else:
                    zt = zpool.tile([C_OUT, W_IMG], f32, name="zt", tag="zt")
                    nc.scalar.activation(
                        zt[:], pst[:],
                        mybir.ActivationFunctionType.Identity,
                        bias=bn[:, 1:2], scale=bn[:, 0:1])
                    nc.vector.scalar_tensor_tensor(
                        ot[:, seg], zt[:], LEAK, zt[:],
                        op0=mybir.AluOpType.mult, op1=mybir.AluOpType.max)
                nc.vector.tensor_tensor(ot[:, seg], ot[:, seg], mt[:, mseg],
                                        op=mybir.AluOpType.mult)
                if y % SG == SG - 1:
                    d0 = (y - SG + 1) * W_IMG
                    nc.scalar.dma_start(out=out_d[:, d0:d0 + SG * W_IMG],
                                        in_=ot[:])

            # rows processed in pairs: the three K=64 dy=2 taps of row y run
            # on PE rows 0-63 (copy0) while row y+1's run on rows 64-127
            # (copy1, same flat offset) — disjoint row-groups + separate PSUM
            # banks execute concurrently, so 2 rows cost 9 MM slots, not 12
            for y in range(0, HS, 2):
                if y % SG == 0:
                    ot = opool.tile([C_OUT, SG * W_IMG], bf, name="ot", tag="ot")
                ps_a = ppool.tile([C_OUT, W_IMG], f32, name="ps_a", tag="pst")
                ps_b = ppool.tile([C_OUT, W_IMG], f32, name="ps_b", tag="pst")
                for yy, ps in ((y, ps_a), (y + 1, ps_b)):
                    for dx in range(3):
                        off = yy * WP + dx
                        nc.tensor.matmul(ps[:], wts[:, dx * 128:dx * 128 + 128],
                                         xs[:, off:off + W_IMG],
                                         start=(dx == 0), stop=False)
                for dx in range(3):
                    off = (y + 2) * WP + dx
                    j = 3 + dx
                    nc.tensor.matmul(ps_a[:], wts[0:64, j * 128:j * 128 + 128],
                                     xs[0:64, off:off + W_IMG],
                                     start=False, stop=(dx == 2))
                    nc.tensor.matmul(ps_b[:], wts[64:128, j * 128:j * 128 + 128],
                                     xs[64:128, off:off + W_IMG],
                                     start=False, stop=(dx == 2))
                epilogue(y, ps_a)
                epilogue(y + 1, ps_b)
    nc.compile()
    return nc


def _get_program(hw_lrelu: bool = True) -> bass.Bass:
    key = ("nc", hw_lrelu)
    if key not in _CACHE:
        _CACHE[key] = _build_program(hw_lrelu)
    return _CACHE[key]


def make_in_maps(x, W, gamma, beta, mean, var, mask):
    """Host-side shard/pack of full inputs into per-core in_maps."""
    x = np.asarray(x, np.float32)
    W = np.asarray(W, np.float32)
    gamma = np.asarray(gamma, np.float32)
    beta = np.asarray(beta, np.float32)
    mean = np.asarray(mean, np.float32)
    var = np.asarray(var, np.float32)
    mask = np.asarray(mask)

    xp = np.pad(x[0], ((0, 0), (1, 1), (1, 1)), mode="reflect")   # [64,514,514]
    xpb = xp.astype(bf16)

    wp = np.zeros((6, 128, C_OUT), np.float32)
    for dx in range(3):
        wp[dx, 0:64] = W[:, :, 0, dx].reshape(C_OUT, C_IN).T
        wp[dx, 64:128] = W[:, :, 1, dx].reshape(C_OUT, C_IN).T
        # dy=2 taps duplicated: rows 0-63 serve even rows via copy0,
        # rows 64-127 serve odd rows via copy1 (concurrent row-tiled MMs)
        wp[3 + dx, 0:64] = W[:, :, 2, dx].reshape(C_OUT, C_IN).T
        wp[3 + dx, 64:128] = W[:, :, 2, dx].reshape(C_OUT, C_IN).T
    wpk = np.ascontiguousarray(
        wp.transpose(1, 0, 2)).reshape(128, 6 * C_OUT).astype(bf16)

    inv = 1.0 / np.sqrt(var + EPS)
    bn = np.stack([gamma * inv, beta - mean * gamma * inv],
                  axis=1).astype(np.float32)                      # [128,2]

    m8 = mask[0].astype(np.uint8)                                 # [128,512,512]

    in_maps = []
    for c in range(N_CORES):
        S = np.ascontiguousarray(xpb[:, HS * c:HS * c + NROW, :])  # [64,66,514]
        xs_c = np.zeros((128, FREE), bf16)
        xs_c[0:64] = S.reshape(C_IN, FREE)
        # only the first HEAD rows of copy1 ship from HBM; the device never
        # reads the rest of this half (it is rebuilt on-chip)
        xs_c[64:128, 0:HEAD * WP] = np.ascontiguousarray(
            S[:, 1:1 + HEAD, :]).reshape(C_IN, HEAD * WP)
        mk_c = np.ascontiguousarray(
            m8[:, HS * c:HS * c + HS, :]).reshape(C_OUT, HS * W_IMG)
        in_maps.append(dict(xs=xs_c, wp=wpk, bn=bn, msk=mk_c))
    return in_maps


def kernel(x, W, gamma, beta, mean, var, mask, _trace=False):
    global LAST_RESULTS
    nc = _get_program()
    in_maps = make_in_maps(x, W, gamma, beta, mean, var, mask)
    res = run_bass_kernel_spmd(nc, in_maps, list(range(N_CORES)), trace=_trace)
    LAST_RESULTS = res
    out = np.empty((1, C_OUT, H, W_IMG), np.float32)
    for c in range(N_CORES):
        out[0, :, HS * c:HS * c + HS, :] = np.asarray(
            res.results[c]["out"]).astype(np.float32).reshape(C_OUT, HS, W_IMG)
    return out


# revision 7
# speedup vs baseline: 1.2496x; 1.1444x over previous
"""Trainium2 Bass kernel for: 3x3 conv (reflect pad) + BatchNorm + LeakyReLU + mask.

Input  x:    (1, 64, 512, 512) f32
       W:    (128, 64, 3, 3)   f32
       gamma/beta/mean/var: (128,) f32
       mask: (1, 128, 512, 512) int32 (0/1)
Output (1, 128, 512, 512) f32

Strategy (8 cores, SPMD):
  - Shard H spatially: core c computes output rows [64c, 64c+64).  Each core
    gets TWO bf16 copies of its 66-row input slab stacked into a [128, 66*514]
    SBUF image (copy1 = copy0 shifted down one row).  A K=128 matmul against
    stacked weights computes two conv taps at once:
      partitions   0..63 : channel ci at row y+dy      (copy0)
      partitions 64..127 : channel ci at row y+dy+1    (copy1)
  - 9 taps -> per row pair: 6 K=128 "pair" matmuls (dy=0&1) plus 6 K=64 dy=2
    matmuls on disjoint partition halves that execute concurrently
    -> 9 N=512 matmul slots per 2 rows (PE-optimal, ~1.92 us/pair).
  - Engine budget per pair must stay under the 1.92 us PE pace: ACT does only
    the two Lrelu activations (~1.55 us), DVE only the two mask multiplies
    (~1.48 us).  Hence NO steady-state DMA triggers on ACT: stores ride the
    sync ring.
  - DMA split (HWDGE bandwidth under contention is shared per-descriptor, so
    small-line tensors starve behind fat ones — keep the critical path at the
    head of its own FIFO):
      sync(qSP):    wts+bn first, then copy0 chunks, then the bf16 stores
      scalar(qAct): copy1 chunks (triggered before the first ACTIVATE)
      gpsimd(q0):   mask chunks
  - Output stored as bf16 (8 MB vs 16) and widened to f32 on the host.
  - PE pre-warm: throwaway matmuls on a zeroed tile while inputs stream in,
    so the HAM clock gate (1.2 -> 2.4 GHz after ~3.4 us busy) is mostly
    released when the real matmul stream starts.
"""

import numpy as np
import ml_dtypes

import concourse.bacc as bacc
import concourse.bass as bass
import concourse.mybir as mybir
import concourse.tile as tile
from concourse.bass_utils import run_bass_kernel_spmd

bf16 = ml_dtypes.bfloat16

N_CORES = 8
C_IN = 64
C_OUT = 128
H = 512
W_IMG = 512
HS = H // N_CORES            # 64 output rows per core
WP = W_IMG + 2               # 514 padded columns
NROW = HS + 2                # 66 input rows per core (64 + 1-row halo each side)
FREE = NROW * WP             # per-partition free elems of the x image
G = 8                        # output rows per mask load chunk
SG = 4                       # output rows per store tile
LEAK = 0.01
EPS = 1e-5

# copy0 load chunks (input row ranges), first chunks small for fast PE start
XCH = [(0, 5), (5, 11), (11, 21), (21, 31), (31, 41), (41, 51), (51, 61), (61, 66)]
# copy1 load chunks (copy1 row r = input row r+1; rows 0..64 are read)
YCH = [(0, 5), (5, 11), (11, 21), (21, 31), (31, 41), (41, 51), (51, 61), (61, 65)]

_CACHE = {}
LAST_RESULTS = None          # BassKernelResults of the last run (for test.py)


def _build_program(hw_lrelu: bool = True) -> bass.Bass:
    """hw_lrelu=True uses the ACT engine's native Lrelu (not implemented in
    CoreSim); False uses an Identity + DVE max(z*a, z) fallback."""
    nc = bacc.Bacc("TRN2", target_bir_lowering=False, debug=False,
                   num_devices=N_CORES)
    f32 = mybir.dt.float32
    bf = mybir.dt.bfloat16
    u8 = mybir.dt.uint8

    xs_d = nc.dram_tensor("xs", [128, FREE], bf, kind="ExternalInput")
    wp_d = nc.dram_tensor("wp", [128, 6 * C_OUT], bf, kind="ExternalInput")
    bn_d = nc.dram_tensor("bn", [C_OUT, 2], f32, kind="ExternalInput")
    mk_d = nc.dram_tensor("msk", [C_OUT, HS * W_IMG], u8, kind="ExternalInput")
    out_d = nc.dram_tensor("out", [C_OUT, HS * W_IMG], bf, kind="ExternalOutput")

    with tile.TileContext(nc) as tc:
        with tc.tile_pool(name="const", bufs=1) as cpool, \
             tc.tile_pool(name="xp", bufs=1) as xpool, \
             tc.tile_pool(name="zp", bufs=4) as zpool, \
             tc.tile_pool(name="op", bufs=6) as opool, \
             tc.tile_pool(name="ps", bufs=8, space="PSUM") as ppool:

            wts = cpool.tile([128, 6 * C_OUT], bf, name="wts", tag="wts")
            bn = cpool.tile([C_OUT, 2], f32, name="bn_t", tag="bn_t")
            mt = cpool.tile([C_OUT, HS * W_IMG], u8, name="mt", tag="mt")
            warm = cpool.tile([128, 128 + W_IMG], bf, name="warm", tag="warm")
            xs = xpool.tile([128, FREE], bf, name="xs_t", tag="xs_t")

            # PE pre-warm on a zeroed tile (see module docstring); the warm
            # psum is just the first rotation of the shared "pst" buffers
            nc.gpsimd.memset(warm[:], 0.0)
            psw = ppool.tile([C_OUT, W_IMG], f32, name="psw", tag="pst")
            for _ in range(6):
                nc.tensor.matmul(psw[:], warm[:, 0:128], warm[:, 128:128 + W_IMG],
                                 start=True, stop=True)

            # sync(qSP) ring: weights first (full queue share -> lands early),
            # then the copy0 stream; stores are appended inside the loop
            nc.sync.dma_start(out=wts[:], in_=wp_d[:])
            nc.sync.dma_start(out=bn[:], in_=bn_d[:])
            for r0, r1 in XCH:
                nc.sync.dma_start(out=xs[0:64, r0 * WP:r1 * WP],
                                  in_=xs_d[0:64, r0 * WP:r1 * WP])

            # scalar(qAct) ring: copy1 stream; all 8 triggers run before the
            # first ACTIVATE, so the ACT engine pays nothing in steady state
            for r0, r1 in YCH:
                nc.scalar.dma_start(out=xs[64:128, r0 * WP:r1 * WP],
                                    in_=xs_d[64:128, r0 * WP:r1 * WP])

            # gpsimd SWDGE: mask chunks on their own queue
            for g in range(HS // G):
                nc.gpsimd.dma_start(
                    out=mt[:, g * G * W_IMG:(g + 1) * G * W_IMG],
                    in_=mk_d[:, g * G * W_IMG:(g + 1) * G * W_IMG])

            ot = None

            def epilogue(y, pst):
                seg = slice((y % SG) * W_IMG, (y % SG + 1) * W_IMG)
                mseg = slice(y * W_IMG, (y + 1) * W_IMG)
                if hw_lrelu:
                    nc.scalar.activation(
                        ot[:, seg], pst[:],
                        mybir.ActivationFunctionType.Lrelu,
                        bias=bn[:, 1:2], scale=bn[:, 0:1], alpha=LEAK)
                else:
                    zt = zpool.tile([C_OUT, W_IMG], f32, name="zt", tag="zt")
                    nc.scalar.activation(
                        zt[:], pst[:],
                        mybir.ActivationFunctionType.Identity,
                        bias=bn[:, 1:2], scale=bn[:, 0:1])
                    nc.vector.scalar_tensor_tensor(
                        ot[:, seg], zt[:], LEAK, zt[:],
                        op0=mybir.AluOpType.mult, op1=mybir.AluOpType.max)
                nc.vector.tensor_tensor(ot[:, seg], ot[:, seg], mt[:, mseg],
                                        op=mybir.AluOpType.mult)
                # stores ride the sync ring (NOT the ACT engine).  The final
                # group is stored in two halves to shorten the serial tail.
                last = (y == HS - 1)
                if y % SG == SG - 1 and not last:
                    d0 = (y - SG + 1) * W_IMG
                    nc.sync.dma_start(out=out_d[:, d0:d0 + SG * W_IMG],
                                      in_=ot[:])
                elif last:
                    d0 = (y - 3) * W_IMG
                    nc.sync.dma_start(
                        out=out_d[:, d0:d0 + 2 * W_IMG], in_=ot[:, 0:2 * W_IMG])
                    nc.sync.dma_start(
                        out=out_d[:, d0 + 2 * W_IMG:d0 + 4 * W_IMG],
                        in_=ot[:, 2 * W_IMG:4 * W_IMG])

            # rows processed in pairs: the three K=64 dy=2 taps of row y run
            # on PE rows 0-63 (copy0) while row y+1's run on rows 64-127
            # (copy1, same flat offset) — disjoint row-groups + separate PSUM
            # banks execute concurrently, so 2 rows cost 9 MM slots, not 12
            for y in range(0, HS, 2):
                if y % SG == 0:
                    ot = opool.tile([C_OUT, SG * W_IMG], bf, name="ot", tag="ot")
                ps_a = ppool.tile([C_OUT, W_IMG], f32, name="ps_a", tag="pst")
                ps_b = ppool.tile([C_OUT, W_IMG], f32, name="ps_b", tag="pst")
                for yy, ps in ((y, ps_a), (y + 1, ps_b)):
                    for dx in range(3):
                        off = yy * WP + dx
                        nc.tensor.matmul(ps[:], wts[:, dx * 128:dx * 128 + 128],
                                         xs[:, off:off + W_IMG],
                                         start=(dx == 0), stop=False)
                for dx in range(3):
                    off = (y + 2) * WP + dx
                    j = 3 + dx
                    nc.tensor.matmul(ps_a[:], wts[0:64, j * 128:j * 128 + 128],
                                     xs[0:64, off:off + W_IMG],
                                     start=False, stop=(dx == 2))
                    nc.tensor.matmul(ps_b[:], wts[64:128, j * 128:j * 128 + 128],
                                     xs[64:128, off:off + W_IMG],
                                     start=False, stop=(dx == 2))
                epilogue(y, ps_a)
                epilogue(y + 1, ps_b)
    nc.compile()
    return nc


def _get_program(hw_lrelu: bool = True) -> bass.Bass:
    key = ("nc", hw_lrelu)
    if key not in _CACHE:
        _CACHE[key] = _build_program(hw_lrelu)
    return _CACHE[key]


def make_in_maps(x, W, gamma, beta, mean, var, mask):
    """Host-side shard/pack of full inputs into per-core in_maps."""
    x = np.asarray(x, np.float32)
    W = np.asarray(W, np.float32)
    gamma = np.asarray(gamma, np.float32)
    beta = np.asarray(beta, np.float32)
    mean = np.asarray(mean, np.float32)
    var = np.asarray(var, np.float32)
    mask = np.asarray(mask)

    xp = np.pad(x[0], ((0, 0), (1, 1), (1, 1)), mode="reflect")   # [64,514,514]
    xpb = xp.astype(bf16)

    wp = np.zeros((6, 128, C_OUT), np.float32)
    for dx in range(3):
        wp[dx, 0:64] = W[:, :, 0, dx].reshape(C_OUT, C_IN).T
        wp[dx, 64:128] = W[:, :, 1, dx].reshape(C_OUT, C_IN).T
        # dy=2 taps duplicated: rows 0-63 serve even rows via copy0,
        # rows 64-127 serve odd rows via copy1 (concurrent row-tiled MMs)
        wp[3 + dx, 0:64] = W[:, :, 2, dx].reshape(C_OUT, C_IN).T
        wp[3 + dx, 64:128] = W[:, :, 2, dx].reshape(C_OUT, C_IN).T
    wpk = np.ascontiguousarray(
        wp.transpose(1, 0, 2)).reshape(128, 6 * C_OUT).astype(bf16)

    inv = 1.0 / np.sqrt(var + EPS)
    bn = np.stack([gamma * inv, beta - mean * gamma * inv],
                  axis=1).astype(np.float32)                      # [128,2]

    m8 = mask[0].astype(np.uint8)                                 # [128,512,512]

    in_maps = []
    for c in range(N_CORES):
        S = np.ascontiguousarray(xpb[:, HS * c:HS * c + NROW, :])  # [64,66,514]
        xs_c = np.zeros((128, FREE), bf16)
        xs_c[0:64] = S.reshape(C_IN, FREE)
        # copy1 row r = input row r+1; rows 0..64 are read by the kernel
        xs_c[64:128, 0:(NROW - 1) * WP] = np.ascontiguousarray(
            S[:, 1:NROW, :]).reshape(C_IN, (NROW - 1) * WP)
        mk_c = np.ascontiguousarray(
            m8[:, HS * c:HS * c + HS, :]).reshape(C_OUT, HS * W_IMG)
        in_maps.append(dict(xs=xs_c, wp=wpk, bn=bn, msk=mk_c))
    return in_maps


def kernel(x, W, gamma, beta, mean, var, mask, _trace=False):
    global LAST_RESULTS
    nc = _get_program()
    in_maps = make_in_maps(x, W, gamma, beta, mean, var, mask)
    res = run_bass_kernel_spmd(nc, in_maps, list(range(N_CORES)), trace=_trace)
    LAST_RESULTS = res
    out = np.empty((1, C_OUT, H, W_IMG), np.float32)
    for c in range(N_CORES):
        out[0, :, HS * c:HS * c + HS, :] = np.asarray(
            res.results[c]["out"]).astype(np.float32).reshape(C_OUT, HS, W_IMG)
    return out
